# revision 13
# baseline (speedup 1.0000x reference)
"""Trainium2 Bass kernel for additive (Bahdanau) attention GNN message passing.

score[n, m] = v . tanh(a[n] + b[m]);  w = softmax(score, axis=n)
ctx[m] = w[:, m].T @ x1;  out = tanh(concat([att, ctx_s, ctx_e]) @ W_lin.T + b)

tanh is replaced by a separable harmonic expansion
  tanh(s) ~ sum_k alpha_k sin(k w0 s),  k in {1,2,3,4,6}
so the O(N*M*H) nonlinearity becomes PE matmuls contracting h.  The HW ACT
Sin table is only valid for |arg| <~ 3.2, so the a-side basis is built from
three in-range ACT sins  s1 = sin(w0 a), c1 = sin(w0 a + pi/2),
s2 = sin(2 w0 a)  plus Squares and a short bf16 product chain; higher
harmonics are expanded as polynomials in these tiles and every
softmax-invariant constant term is dropped, giving 10 rhs "slots":
  s1, c1, s2, qs1=s1^2, qq=qs1^2, P23=s2*qs1, s3p=s1-4/3*P13, c3=c1-4*P31,
  s6p=c3*s3p, qc3=c3^2     (P13=s1*qs1, P31=c1*qs1)
with per-slot b-side lhsT combos  sum_j v*beta_j*(b harmonic tile)  folded
on the small side (beta absorbs alpha_k and expansion coefficients; the
v*beta columns come from a rank-1 PE matmul of single-row DMA constants).
Scores accumulate m-stationary into PSUM [m, n] via 30 512-wide matmuls
(start=True only on each bank's first matmul - start clears the whole
bank's has_written bits).  E = exp(score) is PE-transposed back to [n, m]
for the ctx matmuls; softmax sums fall out of a ones column in the x image.
Inputs arrive as packed bf16 images spread over 3 DMA queues (per-queue
DMA bandwidth is ~77GB/s, so parallelism and few triggers matter).
"""

import numpy as np
from ml_dtypes import bfloat16

import concourse.bass as bass
import concourse.tile as tile
from concourse import bacc, masks, mybir
from concourse.bass_utils import run_bass_kernel_spmd

F32 = mybir.dt.float32
F16 = mybir.dt.float16
BF16 = mybir.dt.bfloat16
AF = mybir.ActivationFunctionType
OP = mybir.AluOpType

H = 128
A = 256
N_S = 1024
N_E = 512
M = 1024
NC = 8
ML = M // NC
NT = N_S + N_E
NCH = NT // 128
CW = 129
X16W = NCH * CW          # 1548
IMG2W = X16W + 3 * A     # x16 | wlinT16

W0 = 0.267059
AL = {1: 1.17663, 2: 0.12087, 3: 0.17747, 4: 0.13768, 6: 0.13409}

# slot -> terms (k, trig, coef): lhsT = sum_j v * coef_j*alpha_kj * btile
# trig 0 => pairs the b-side cos tile, 1 => the b-side sin tile
SLOTS = (
    ("s1",  ((1, 0, 1.0),)),
    ("c1",  ((1, 1, 1.0),)),
    ("s2",  ((2, 0, 1.0), (4, 0, 2.0))),
    ("qs1", ((2, 1, -2.0), (4, 1, -8.0))),
    ("qq",  ((4, 1, 8.0),)),
    ("P23", ((4, 0, -4.0),)),
    ("s3p", ((3, 0, 3.0),)),
    ("c3",  ((3, 1, 1.0),)),
    ("s6p", ((6, 0, 6.0),)),
    ("qc3", ((6, 1, 2.0),)),
)
BETAS = [coef * AL[k] for _, terms in SLOTS for k, _, coef in terms]  # 12

PARTS = ((0, 512), (512, 1536))
BLOCKS = ((0, 512, 0), (512, 1024, 0), (1024, 1536, 1))  # (lo, hi, set)

_CACHE = {}


def _build():
    nc = bacc.Bacc(
        "TRN2", target_bir_lowering=False, debug=False, num_devices=NC
    )
    dr = lambda nm, sh: nc.dram_tensor(nm, sh, BF16, kind="ExternalInput").ap()
    d_imgC = dr("imgC", [128, 384])     # W2s.T | W2e.T | attT16
    d_imgD = dr("imgD", [128, 256])     # W1s.T | W1e.T
    d_b1 = dr("b1", [128, 512])         # stmtsT[:, 0:512]
    d_b2 = dr("b2", [128, 512])         # stmtsT[:, 512:1024]
    d_b3 = dr("b3", [128, 512])         # eresT
    d_crow = dr("crow", [1, 1536])      # vs|ve|bcs|bce|betas|blin
    d_img2 = dr("img2", [128, IMG2W])   # x16 | wlinT16
    d_out = nc.dram_tensor("out", [ML, A], F16, kind="ExternalOutput").ap()

    with tile.TileContext(nc) as tc:
        _emit(nc, tc, d_imgC, d_imgD, d_b1, d_b2, d_b3, d_crow, d_img2, d_out)

    nc.compile()
    return nc


def _emit(nc, tc, d_imgC, d_imgD, d_b1, d_b2, d_b3, d_crow, d_img2, d_out):
    from contextlib import ExitStack

    ctx = ExitStack()
    with ctx:
        const = ctx.enter_context(tc.tile_pool(name="const", bufs=1))
        bpool = ctx.enter_context(tc.tile_pool(name="bpool", bufs=1))
        apool = ctx.enter_context(tc.tile_pool(name="apool", bufs=1))
        ps_a = ctx.enter_context(
            tc.tile_pool(name="ps_a", bufs=1, space=bass.MemorySpace.PSUM))
        ps_score = ctx.enter_context(
            tc.tile_pool(name="ps_score", bufs=1, space=bass.MemorySpace.PSUM))
        ps_small = ctx.enter_context(
            tc.tile_pool(name="ps_small", bufs=1, space=bass.MemorySpace.PSUM))

        # ---- init + table warm ----
        ident16 = const.tile([128, 128], BF16)
        masks.make_identity(nc, ident16[:])
        ones16 = const.tile([1, 128], BF16)
        nc.gpsimd.memset(ones16[:], 1.0)
        pz = const.tile([128, 1], F32)
        nc.gpsimd.memset(pz[:], 1.5707963267948966)
        scratch = const.tile([128, 1], F32)
        nc.gpsimd.memset(scratch[:], 0.0)

        # ---- DMAs spread over 3 queues (scalar-queue triggers first so the
        # ACT table hoisting isn't split around them) ----
        sb_imgD = const.tile([128, 256], BF16)
        nc.scalar.dma_start(sb_imgD[:], d_imgD[:, :])
        sb_b1 = const.tile([128, 512], BF16)
        nc.scalar.dma_start(sb_b1[:], d_b1[:, :])
        nc.scalar.activation(scratch[:], scratch[:], AF.Sin)
        sb_crow = const.tile([1, 1536], BF16)
        nc.sync.dma_start(sb_crow[0:1, :], d_crow[0:1, :])
        sb_imgC = const.tile([128, 384], BF16)
        nc.sync.dma_start(sb_imgC[:], d_imgC[:, :])
        sb_b2 = const.tile([128, 512], BF16)
        nc.gpsimd.dma_start(sb_b2[:], d_b2[:, :])
        sb_b3 = const.tile([128, 512], BF16)
        nc.gpsimd.dma_start(sb_b3[:], d_b3[:, :])
        sb_img2 = const.tile([128, IMG2W], BF16)
        nc.gpsimd.dma_start(sb_img2[:], d_img2[:, :])

        attT16 = sb_imgC[:, 256:384]
        crow = lambda r, n: sb_crow[0:1, r * 256:r * 256 + n]
        x16 = sb_img2[:, 0:X16W]
        wlin = lambda j: sb_img2[:, X16W + j * A:X16W + (j + 1) * A]

        # ---- front PE: bT + bias + v*beta rank-1, one PSUM bank ----
        NB = len(BETAS)
        ps_bT = ps_small.tile([128, 288], F32, tag="ctx", name="ps_bT")
        nc.tensor.matmul(ps_bT[:, 256:256 + NB], crow(0, 128), crow(4, NB),
                         start=True, stop=False, skip_group_check=True)
        nc.tensor.matmul(ps_bT[:, 256 + NB:256 + 2 * NB], crow(1, 128),
                         crow(4, NB), start=False, stop=False,
                         skip_group_check=True)
        sb_vbeta = const.tile([128, 2 * NB], F32)
        nc.vector.tensor_copy(sb_vbeta[:], ps_bT[:, 256:256 + 2 * NB])
        nc.tensor.matmul(ps_bT[:, 0:ML], crow(2, 128), ones16[0:1, :],
                         start=False, stop=False, skip_group_check=True)
        nc.tensor.matmul(ps_bT[:, ML:2 * ML], crow(3, 128), ones16[0:1, :],
                         start=False, stop=False, skip_group_check=True)
        nc.tensor.matmul(ps_bT[:, 0:ML], sb_imgC[:, 0:128], attT16,
                         start=False, stop=False, skip_group_check=True)
        nc.tensor.matmul(ps_bT[:, ML:2 * ML], sb_imgC[:, 128:256], attT16,
                         start=False, stop=True, skip_group_check=True)

        # ---- aT -> [128, 1536] PSUM ----
        ps_aT = ps_a.tile([128, NT], F32, tag="aT", name="ps_aT")
        nc.tensor.matmul(ps_aT[:, 0:512], sb_imgD[:, 0:128], sb_b1[:],
                         start=True, stop=True)
        nc.tensor.matmul(ps_aT[:, 512:1024], sb_imgD[:, 0:128], sb_b2[:],
                         start=True, stop=True)
        nc.tensor.matmul(ps_aT[:, 1024:1536], sb_imgD[:, 128:256], sb_b3[:],
                         start=True, stop=True)

        # att + b_lin parts of the final linear
        ps_out = ps_small.tile([128, 400], F32, tag="out")
        nc.tensor.matmul(ps_out[:, 0:A], attT16, wlin(0),
                         start=True, stop=False, skip_group_check=True)
        nc.tensor.matmul(ps_out[:, 0:A], ones16[0:1, :], crow(5, A),
                         start=False, stop=False, skip_group_check=True)

        # ---- b-side basis (small): ACT sins + Pool/DVE chain ----
        # u1b reads cols 0:280 (incl. junk v*beta cols) so ACT's first
        # PSUM-bank read orders after all PE writes (collision avoidance)
        u1b = bpool.tile([128, 288], BF16, name="u1b")
        nc.scalar.activation(u1b[:, 0:280], ps_bT[:, 0:280], AF.Sin,
                             scale=0.5 * W0)
        bt = {k: bpool.tile([128, 512], BF16, name=f"bt{k}") for k in AL}
        nc.scalar.activation(bt[1][:, 0:256], ps_bT[:, 0:256], AF.Sin,
                             scale=W0)
        s1b = bt[1][:, 0:256]

        def btmp(nm):
            return bpool.tile([128, 256], BF16, name=nm)[:]

        q1b = btmp("q1b")
        nc.vector.tensor_tensor(q1b, u1b[:, 0:256], u1b[:, 0:256], OP.mult)
        c1b = bt[1][:, 256:512]
        nc.vector.tensor_scalar(c1b, q1b, -2.0, 1.0, OP.mult, OP.add)
        c1twob = btmp("c1twob")
        nc.vector.tensor_scalar_mul(c1twob, c1b, 2.0)
        nc.vector.tensor_tensor(bt[2][:, 0:256], c1twob, s1b, OP.mult)  # s2b
        qs1b = btmp("qs1b")
        nc.vector.tensor_tensor(qs1b, s1b, s1b, OP.mult)
        c2b = bt[2][:, 256:512]
        nc.vector.tensor_scalar(c2b, qs1b, -2.0, 1.0, OP.mult, OP.add)
        c2twob = btmp("c2twob")
        nc.vector.tensor_scalar_mul(c2twob, c2b, 2.0)
        c2mb = btmp("c2mb")
        nc.vector.tensor_scalar_sub(c2mb, c2twob, 1.0)
        nc.vector.tensor_tensor(bt[3][:, 256:512], c1b, c2mb, OP.mult)  # c3b
        s3tb = btmp("s3tb")
        nc.vector.tensor_tensor(s3tb, c1twob, bt[2][:, 0:256], OP.mult)
        nc.vector.tensor_tensor(bt[3][:, 0:256], s3tb, s1b, OP.subtract)  # s3b
        nc.vector.tensor_tensor(bt[4][:, 0:256], c2twob, bt[2][:, 0:256],
                                OP.mult)                            # s4b
        qs2b = btmp("qs2b")
        nc.vector.tensor_tensor(qs2b, bt[2][:, 0:256], bt[2][:, 0:256],
                                OP.mult)
        nc.vector.tensor_scalar(bt[4][:, 256:512], qs2b, -2.0, 1.0,
                                OP.mult, OP.add)                    # c4b
        c3twob = btmp("c3twob")
        nc.vector.tensor_scalar_mul(c3twob, bt[3][:, 256:512], 2.0)
        nc.vector.tensor_tensor(bt[6][:, 0:256], c3twob, bt[3][:, 0:256],
                                OP.mult)                            # s6b
        qc3b = btmp("qc3b")
        nc.vector.tensor_tensor(qc3b, bt[3][:, 256:512], bt[3][:, 256:512],
                                OP.mult)
        nc.vector.tensor_scalar(bt[6][:, 256:512], qc3b, 2.0, -1.0,
                                OP.mult, OP.add)                    # c6b

        # ---- slot lhsT combos: w = sum_j (v*beta_j) * btile_j ----
        # single terms on Pool ((AP, imm, mult, mult) form), second term of
        # the 2-term combos via DVE scalar_tensor_tensor with an AP scalar
        wsl = {}
        bi = 0
        for nm, terms in SLOTS:
            wsl[nm] = bpool.tile([128, 256], BF16, name=f"w_{nm}")
            for st in range(2):
                k0, tr0, _ = terms[0]
                src0 = bt[k0][:, (1 - tr0) * 256 + st * 128:
                              (1 - tr0) * 256 + st * 128 + 128]
                dst = wsl[nm][:, st * 128:st * 128 + 128]
                if len(terms) == 1:
                    nc.vector.tensor_scalar(
                        dst, src0,
                        sb_vbeta[:, st * NB + bi:st * NB + bi + 1],
                        None, OP.mult)
                else:
                    t0 = bpool.tile([128, 128], BF16, name=f"wt_{nm}{st}")
                    nc.vector.tensor_scalar(
                        t0[:], src0,
                        sb_vbeta[:, st * NB + bi:st * NB + bi + 1],
                        None, OP.mult)
                    k1_, tr1, _ = terms[1]
                    src1 = bt[k1_][:, (1 - tr1) * 256 + st * 128:
                                   (1 - tr1) * 256 + st * 128 + 128]
                    nc.vector.scalar_tensor_tensor(
                        dst, src1,
                        sb_vbeta[:, st * NB + bi + 1:st * NB + bi + 2],
                        t0[:], OP.mult, OP.add)
            bi += len(terms)

        # ---- a-side basis ----
        at = {}
        for nm in ("s1", "c1", "s2", "qs1", "qq", "P13", "P31", "P23",
                   "s3p", "c3", "s6p", "qc3"):
            at[nm] = apool.tile([128, NT], BF16, name=nm)

        def act1(out, in_, func, p, scale=1.0, bias=0.0):
            lo, hi = PARTS[p]
            nc.scalar.activation(out[:, lo:hi], in_[:, lo:hi], func,
                                 scale=scale, bias=bias)

        def dve_tt1(out, in0, in1, op, p):
            lo, hi = PARTS[p]
            nc.vector.tensor_tensor(out[:, lo:hi], in0[:, lo:hi],
                                    in1[:, lo:hi], op)

        def dve_stt1(out, in0, sc, in1, op0, op1, p):
            lo, hi = PARTS[p]
            nc.vector.scalar_tensor_tensor(out[:, lo:hi], in0[:, lo:hi],
                                           sc, in1[:, lo:hi], op0, op1)

        # ACT: part-0 sins first so the part-0 DVE chain starts early
        for p in range(2):
            act1(at["s1"][:], ps_aT[:], AF.Sin, p, W0)
            act1(at["c1"][:], ps_aT[:], AF.Sin, p, W0, pz[:, 0:1])
            act1(at["qs1"][:], at["s1"][:], AF.Square, p)
            act1(at["s2"][:], ps_aT[:], AF.Sin, p, 2 * W0)
        for p in range(2):
            act1(at["qq"][:], at["qs1"][:], AF.Square, p)
        # DVE products: full part-0 chain, then part-1
        for p in range(2):
            lo, hi = PARTS[p]
            nc.gpsimd.tensor_tensor(at["P13"][:, lo:hi], at["s1"][:, lo:hi],
                                    at["qs1"][:, lo:hi], OP.mult)
            dve_tt1(at["P31"][:], at["c1"][:], at["qs1"][:], OP.mult, p)
            nc.gpsimd.tensor_tensor(at["P23"][:, lo:hi], at["s2"][:, lo:hi],
                                    at["qs1"][:, lo:hi], OP.mult)
            dve_stt1(at["c3"][:], at["P31"][:], -4.0, at["c1"][:],
                     OP.mult, OP.add, p)
            dve_stt1(at["s3p"][:], at["P13"][:], -4.0 / 3.0, at["s1"][:],
                     OP.mult, OP.add, p)
            act1(at["qc3"][:], at["c3"][:], AF.Square, p)
            dve_tt1(at["s6p"][:], at["c3"][:], at["s3p"][:], OP.mult, p)

        # ---- scores, m-stationary: ps_sc[m, n] ----
        ps_sc = ps_score.tile([128, NT], F32)
        for lo, hi, st in BLOCKS:
            for si, (nm, _) in enumerate(SLOTS):
                nc.tensor.matmul(
                    ps_sc[:, lo:hi], wsl[nm][:, st * 128:st * 128 + 128],
                    at[nm][:, lo:hi],
                    start=(si == 0), stop=(si == len(SLOTS) - 1),
                    skip_group_check=True)

        # ---- epilogue ----
        E_mT = apool.tile([128, NT], BF16, name="E_mT")
        for lo, hi, _ in BLOCKS:
            nc.scalar.activation(E_mT[:, lo:hi], ps_sc[:, lo:hi], AF.Exp)

        ps_tr2 = ps_a.tile([128, 2 * NT], BF16, tag="aT", name="ps_tr2")
        sb_E = apool.tile([128, NT], BF16, name="sb_E")
        for c in range(NCH):
            nc.tensor.matmul(ps_tr2[:, c * 128:(c + 1) * 128],
                             E_mT[:, c * 128:(c + 1) * 128], ident16[:],
                             is_transpose=True)
            if c == 7:
                nc.vector.tensor_copy(sb_E[:, 0:1024], ps_tr2[:, 0:1024])
        nc.vector.tensor_copy(sb_E[:, 1024:1536], ps_tr2[:, 1024:1536])

        ps_ctx = ps_small.tile([128, 288], F32, tag="ctx", name="ps_ctx")
        for c in range(8):
            nc.tensor.matmul(ps_ctx[:, 0:CW],
                             sb_E[:, c * 128:(c + 1) * 128],
                             x16[:, c * CW:(c + 1) * CW],
                             start=(c == 0), stop=(c == 7))
        for c in range(8, 12):
            nc.tensor.matmul(ps_out[:, 264:264 + CW],
                             sb_E[:, c * 128:(c + 1) * 128],
                             x16[:, c * CW:(c + 1) * CW],
                             start=False, stop=False, skip_group_check=True)

        sb_recip = apool.tile([128, 2], F32, name="recip")
        nc.vector.reciprocal(sb_recip[:, 0:1], ps_ctx[:, H:H + 1])
        nc.vector.reciprocal(sb_recip[:, 1:2], ps_out[:, 392:393])
        sb_ctx = apool.tile([128, 2 * H], BF16, name="sb_ctx")
        nc.vector.tensor_scalar(sb_ctx[:, 0:H], ps_ctx[:, 0:H],
                                sb_recip[:, 0:1], None, OP.mult)
        nc.vector.tensor_scalar(sb_ctx[:, H:2 * H], ps_out[:, 264:392],
                                sb_recip[:, 1:2], None, OP.mult)

        ps_tr3 = ps_a.tile([128, 2 * NT], BF16, tag="aT", name="ps_tr3")
        nc.tensor.matmul(ps_tr3[:, 0:128], sb_ctx[:, 0:H], ident16[:],
                         is_transpose=True)
        nc.tensor.matmul(ps_tr3[:, 128:256], sb_ctx[:, H:2 * H], ident16[:],
                         is_transpose=True)
        sb_ctxT = apool.tile([128, 2 * H], BF16, name="sb_ctxT")
        nc.vector.tensor_copy(sb_ctxT[:], ps_tr3[:, 0:256])

        nc.tensor.matmul(ps_out[:, 0:A], sb_ctxT[:, 0:H], wlin(1),
                         start=False, stop=False, skip_group_check=True)
        nc.tensor.matmul(ps_out[:, 0:A], sb_ctxT[:, H:2 * H], wlin(2),
                         start=False, stop=True, skip_group_check=True)

        sb_out = apool.tile([128, A], F16, name="sb_out")
        nc.scalar.activation(sb_out[:], ps_out[:, 0:A], AF.Tanh)
        nc.sync.dma_start(d_out[0:48, :], sb_out[0:48, :])
        nc.scalar.dma_start(d_out[48:96, :], sb_out[48:96, :])
        nc.gpsimd.dma_start(d_out[96:128, :], sb_out[96:128, :])


def _get_nc():
    if "nc" not in _CACHE:
        _CACHE["nc"] = _build()
    return _CACHE["nc"]


def _prep_inputs(inputs):
    """Host-side layout prep: transposes / bf16 casts / image packing."""
    f = {k: np.ascontiguousarray(np.asarray(v, np.float32))
         for k, v in inputs.items()}
    stmts, eres = f["attendee_stmts"], f["attendee_eres"]
    ws, we, wlin = f["Ws_concat"], f["We_concat"], f["W_lin"]

    stT = stmts.T
    imgD = np.concatenate([ws[:, :H].T, we[:, :H].T], axis=1)

    crow = np.zeros((1, 1536), np.float32)
    crow[0, 0:128] = f["vs_single"]
    crow[0, 256:384] = f["ve_single"]
    crow[0, 512:640] = f["bs_concat"]
    crow[0, 768:896] = f["be_concat"]
    crow[0, 1024:1024 + len(BETAS)] = np.asarray(BETAS, np.float32)
    crow[0, 1280:1536] = f["b_lin"]

    x = np.empty((128, X16W), np.float32)
    for c in range(8):
        x[:, c * CW:c * CW + H] = stmts[c * 128:(c + 1) * 128]
        x[:, c * CW + H] = 1.0
    for c in range(8, 12):
        x[:, c * CW:c * CW + H] = eres[(c - 8) * 128:(c - 7) * 128]
        x[:, c * CW + H] = 1.0
    wlinT = np.concatenate(
        [wlin[:, 0:H].T, wlin[:, H:2 * H].T, wlin[:, 2 * H:3 * H].T], axis=1)
    img2 = np.concatenate([x, wlinT], axis=1)

    cb = lambda a_: np.ascontiguousarray(a_.astype(bfloat16))
    shared = {
        "imgD": cb(imgD), "b1": cb(stT[:, 0:512]), "b2": cb(stT[:, 512:1024]),
        "b3": cb(eres.T), "crow": cb(crow), "img2": cb(img2),
    }
    att = f["attender"]
    in_maps = []
    for i in range(NC):
        imgC = np.concatenate(
            [ws[:, H:].T, we[:, H:].T, att[i * ML:(i + 1) * ML].T], axis=1)
        in_maps.append(dict(shared, imgC=cb(imgC)))
    return in_maps


def kernel(**inputs) -> np.ndarray:
    nc = _get_nc()
    in_maps = _prep_inputs(inputs)
    res = run_bass_kernel_spmd(nc, in_maps, list(range(NC)))
    return np.concatenate(
        [res.results[i]["out"].astype(np.float32) for i in range(NC)], axis=0)


# revision 14
# speedup vs baseline: 1.2863x; 1.2863x over previous
"""Trainium2 Bass kernel for additive (Bahdanau) attention GNN message passing.

score[n, m] = v . tanh(a[n] + b[m]),  a = x1 @ W1.T, b = x2 @ W2.T + bc
w = softmax(score, axis=n);  ctx[m] = w[:, m].T @ x1
out = tanh(concat([att, ctx_s, ctx_e]) @ W_lin.T + b_lin)

Sharding: attender dim M=1024 split across 8 cores (128 each); attendees and
params replicated. No collectives.

Key trick: the per-(n,m,h) tanh (25M ACT elems/core in the naive scheme) is
replaced by a separable harmonic expansion
    tanh(s) ~ sum_k alpha_k sin(k w0 s),   s = a + b
    sin(k w0 (a+b)) = sin(k w0 a) cos(k w0 b) + cos(k w0 a) sin(k w0 b)
so the O(N*M*H) work becomes PE matmuls contracting h for each harmonic,
and the nonlinearity cost drops to O((N+M)*H) basis evaluations.

The HW ACT Sin table is only valid for |arg| < ~pi, so only small-argument
sins run on ACT (.5*w0*x, w0*x, 1.5*w0*x; args <= 2.7 rad) plus Squares;
cosines come from cos(2t) = 1 - 2 sin^2(t) and higher harmonics from
bf16 Chebyshev product recurrences on DVE. Weighting v*alpha folds into the
small b-side tiles (gpsimd). Scores accumulate in PSUM over 2F matmuls per
128-attendee chunk; softmax sums fall out of the ctx matmul via a ones
column in the attendee image; final linear runs in f32r.

PSUM accumulation note: start=True clears the has_written bits of the WHOLE
bank, so only the first matmul touching each bank may set it; later
first-writes to other regions rely on per-element overwrite-then-accumulate.
"""

import numpy as np
from ml_dtypes import bfloat16

import concourse.bass as bass
import concourse.tile as tile
from concourse import bacc, masks, mybir
from concourse.bass_utils import run_bass_kernel_spmd

F32 = mybir.dt.float32
F32R = mybir.dt.float32r
BF16 = mybir.dt.bfloat16
AF = mybir.ActivationFunctionType
OP = mybir.AluOpType

H = 128      # hidden
A = 256      # attention (output) size
N_S = 1024   # attendee statements
N_E = 512    # attendee EREs
M = 1024     # attenders
NC = 8       # cores
ML = M // NC # attenders per core
NT = N_S + N_E  # 1536
NCH = NT // 128  # 12 chunks of attendees
CW = 129     # x-image chunk width: 128 attendee cols + a ones column

W0 = 0.267059
KS = (1, 2, 3, 4, 6)
ALPHA = (1.17663, 0.12087, 0.17747, 0.13768, 0.13409)

_CACHE = {}


def _build():
    nc = bacc.Bacc(
        "TRN2", target_bir_lowering=False, debug=False, num_devices=NC
    )

    d_x16 = nc.dram_tensor("x16", [128, NCH * CW], BF16, kind="ExternalInput").ap()
    d_stmtsT = nc.dram_tensor("stmtsT", [128, N_S], BF16, kind="ExternalInput").ap()
    d_eresT = nc.dram_tensor("eresT", [128, N_E], BF16, kind="ExternalInput").ap()
    d_attT16 = nc.dram_tensor("attT16", [128, ML], BF16, kind="ExternalInput").ap()
    d_wT16 = nc.dram_tensor("wT16", [128, 4 * H], BF16, kind="ExternalInput").ap()
    d_attTf = nc.dram_tensor("attTf", [128, ML], F32, kind="ExternalInput").ap()
    d_wlinT = nc.dram_tensor("wlinT", [128, 3 * A], F32, kind="ExternalInput").ap()
    d_vb = nc.dram_tensor("vb", [128, 4], F32, kind="ExternalInput").ap()
    d_blin = nc.dram_tensor("blin", [1, A], F32, kind="ExternalInput").ap()
    d_out = nc.dram_tensor("out", [ML, A], F32, kind="ExternalOutput").ap()

    with tile.TileContext(nc) as tc:
        _emit(nc, tc, d_x16, d_stmtsT, d_eresT, d_attT16, d_wT16,
              d_attTf, d_wlinT, d_vb, d_blin, d_out)

    nc.compile()
    return nc


def _emit(nc, tc, d_x16, d_stmtsT, d_eresT, d_attT16, d_wT16,
          d_attTf, d_wlinT, d_vb, d_blin, d_out):
    from contextlib import ExitStack

    ctx = ExitStack()
    with ctx:
        const = ctx.enter_context(tc.tile_pool(name="const", bufs=1))
        bpool = ctx.enter_context(tc.tile_pool(name="bpool", bufs=1))
        apool = ctx.enter_context(tc.tile_pool(name="apool", bufs=1))
        ps_a = ctx.enter_context(
            tc.tile_pool(name="ps_a", bufs=1, space=bass.MemorySpace.PSUM))
        ps_score = ctx.enter_context(
            tc.tile_pool(name="ps_score", bufs=1, space=bass.MemorySpace.PSUM))
        ps_small = ctx.enter_context(
            tc.tile_pool(name="ps_small", bufs=1, space=bass.MemorySpace.PSUM))

        # ---- gpsimd init + ACT table warm ----
        ident = const.tile([128, 128], F32)
        masks.make_identity(nc, ident[:])
        ones_row = const.tile([1, 128], F32)
        nc.gpsimd.memset(ones_row[:], 1.0)
        scratch = const.tile([128, 1], F32)
        nc.gpsimd.memset(scratch[:], 0.0)
        nc.scalar.activation(scratch[:], scratch[:], AF.Sin)  # load trig table

        # ---- DMAs: front-critical on sync queue, epilogue-only on gpsimd ----
        sb_wT = const.tile([128, 4 * H], BF16)
        nc.sync.dma_start(sb_wT[:], d_wT16[:, :])
        sb_attT16 = const.tile([128, ML], BF16)
        nc.sync.dma_start(sb_attT16[:], d_attT16[:, :])
        sb_vb = const.tile([128, 4], F32)
        nc.sync.dma_start(sb_vb[:], d_vb[:, :])
        sb_stmtsT = const.tile([128, N_S], BF16)
        nc.sync.dma_start(sb_stmtsT[:], d_stmtsT[:, :])
        sb_eresT = const.tile([128, N_E], BF16)
        nc.sync.dma_start(sb_eresT[:], d_eresT[:, :])

        sb_x16 = const.tile([128, NCH * CW], BF16)
        nc.gpsimd.dma_start(sb_x16[:], d_x16[:, :])
        sb_attTf = const.tile([128, ML], F32R)
        nc.gpsimd.dma_start(sb_attTf[:], d_attTf[:, :])
        sb_wlinT = const.tile([128, 3 * A], F32R)
        nc.gpsimd.dma_start(sb_wlinT[:], d_wlinT[:, :])
        sb_blin = const.tile([1, A], F32)
        nc.gpsimd.dma_start(sb_blin[0:1, :], d_blin[0:1, :])

        # ---- front matmuls ----
        # bT for both sets -> one [128, 258] PSUM tile (tag shared with ctx)
        ps_bT = ps_small.tile([128, 2 * CW], F32, tag="ctx", name="ps_bT")
        nc.tensor.matmul(ps_bT[:, 0:ML], sb_wT[:, 128:256], sb_attT16[:],
                         start=True, stop=True)
        nc.tensor.matmul(ps_bT[:, CW:CW + ML], sb_wT[:, 384:512], sb_attT16[:],
                         start=True, stop=True)
        sb_b2 = const.tile([128, 2 * ML], F32)
        nc.vector.tensor_scalar_add(sb_b2[:, 0:ML], ps_bT[:, 0:ML],
                                    sb_vb[:, 2:3])
        nc.vector.tensor_scalar_add(sb_b2[:, ML:2 * ML], ps_bT[:, CW:CW + ML],
                                    sb_vb[:, 3:4])

        # aT for both sets -> one [128, 1536] PSUM tile (bank-aligned pieces)
        ps_aT = ps_a.tile([128, NT], F32, tag="aT", name="ps_aT")
        nc.tensor.matmul(ps_aT[:, 0:512], sb_wT[:, 0:128],
                         sb_stmtsT[:, 0:512], start=True, stop=True)
        nc.tensor.matmul(ps_aT[:, 512:1024], sb_wT[:, 0:128],
                         sb_stmtsT[:, 512:1024], start=True, stop=True)
        nc.tensor.matmul(ps_aT[:, 1024:1536], sb_wT[:, 256:384],
                         sb_eresT[:], start=True, stop=True)

        # att + b_lin parts of the final linear (f32r, off critical path)
        ps_out = ps_small.tile([128, A], F32, tag="out")
        nc.tensor.matmul(ps_out[:], sb_attTf[:], sb_wlinT[:, 0:A],
                         start=True, stop=False, skip_group_check=True)
        nc.tensor.matmul(ps_out[:], ones_row[0:1, :], sb_blin[0:1, :],
                         start=False, stop=False, skip_group_check=True)

        # ---- basis generation helpers ----
        def gen_basis(pool, src_ap, width, name, act_square_s3):
            """Emit sin/cos harmonic tiles of src (bf16, [128, width]).

            Returns dict k -> (sin_tile, cos_tile)."""
            t = {}

            def tl(nm):
                return pool.tile([128, width], BF16, name=f"{nm}_{name}")

            u1, s1, u3 = tl("u1"), tl("s1"), tl("u3")
            nc.scalar.activation(u1[:], src_ap, AF.Sin, scale=0.5 * W0)
            nc.scalar.activation(s1[:], src_ap, AF.Sin, scale=W0)
            nc.scalar.activation(u3[:], src_ap, AF.Sin, scale=1.5 * W0)
            squ1, squ3 = tl("squ1"), tl("squ3")
            nc.scalar.activation(squ1[:], u1[:], AF.Square)
            nc.scalar.activation(squ3[:], u3[:], AF.Square)
            c1, c3 = tl("c1"), tl("c3")
            nc.vector.tensor_scalar(c1[:], squ1[:], -2.0, 1.0, OP.mult, OP.add)
            c1two = tl("c1two")
            nc.vector.tensor_scalar_mul(c1two[:], c1[:], 2.0)
            nc.vector.tensor_scalar(c3[:], squ3[:], -2.0, 1.0, OP.mult, OP.add)
            s2 = tl("s2")
            nc.vector.tensor_tensor(s2[:], c1two[:], s1[:], OP.mult)
            c2t, c2 = tl("c2t"), tl("c2")
            nc.vector.tensor_tensor(c2t[:], c1two[:], c1[:], OP.mult)
            nc.vector.tensor_scalar_sub(c2[:], c2t[:], 1.0)
            c2two = tl("c2two")
            nc.vector.tensor_scalar_mul(c2two[:], c2[:], 2.0)
            s3t, s3 = tl("s3t"), tl("s3")
            nc.vector.tensor_tensor(s3t[:], c1two[:], s2[:], OP.mult)
            nc.vector.tensor_tensor(s3[:], s3t[:], s1[:], OP.subtract)
            s4 = tl("s4")
            nc.vector.tensor_tensor(s4[:], c2two[:], s2[:], OP.mult)
            c4t, c4 = tl("c4t"), tl("c4")
            nc.vector.tensor_tensor(c4t[:], c2two[:], c2[:], OP.mult)
            nc.vector.tensor_scalar_sub(c4[:], c4t[:], 1.0)
            s6t, s6 = tl("s6t"), tl("s6")
            nc.vector.tensor_tensor(s6t[:], c2two[:], s4[:], OP.mult)
            nc.vector.tensor_tensor(s6[:], s6t[:], s2[:], OP.subtract)
            sqs3, c6 = tl("sqs3"), tl("c6")
            if act_square_s3:
                nc.scalar.activation(sqs3[:], s3[:], AF.Square)
            else:
                nc.vector.tensor_tensor(sqs3[:], s3[:], s3[:], OP.mult)
            nc.vector.tensor_scalar(c6[:], sqs3[:], -2.0, 1.0, OP.mult, OP.add)
            t[1] = (s1, c1)
            t[2] = (s2, c2)
            t[3] = (s3, c3)
            t[4] = (s4, c4)
            t[6] = (s6, c6)
            return t

        # b-side basis (small, both sets side by side) — DVE squares for s3
        bt = gen_basis(bpool, sb_b2[:], 2 * ML, "b", act_square_s3=False)

        # b-side weighting: w = bf16(v_set * alpha_k * tile_half), on gpsimd
        wsin = {}
        wcos = {}
        for k, al in zip(KS, ALPHA):
            ws = bpool.tile([128, 2 * ML], BF16, name=f"wsin{k}")
            wc = bpool.tile([128, 2 * ML], BF16, name=f"wcos{k}")
            for half, vcol in ((0, 0), (1, 1)):
                lo = half * ML
                nc.gpsimd.tensor_scalar(ws[:, lo:lo + ML],
                                        bt[k][0][:, lo:lo + ML],
                                        sb_vb[:, vcol:vcol + 1], al,
                                        OP.mult, OP.mult)
                nc.gpsimd.tensor_scalar(wc[:, lo:lo + ML],
                                        bt[k][1][:, lo:lo + ML],
                                        sb_vb[:, vcol:vcol + 1], al,
                                        OP.mult, OP.mult)
            wsin[k] = ws
            wcos[k] = wc

        # a-side basis (big tiles) — ACT square for s3 (balances engines)
        at = gen_basis(apool, ps_aT[:], NT, "a", act_square_s3=True)

        # ---- score matmuls: ps_sT[n_l, (c, m)] += sum_k sa*wcb + ca*wsb ----
        # start=True clears the has_written bits of the WHOLE PSUM bank, so
        # only the first matmul touching each bank (chunks 0/4/8) may set it;
        # later first-writes to other chunk regions rely on the per-element
        # overwrite-then-accumulate semantics.
        ps_sT = ps_score.tile([128, NT], F32)
        for ki, k in enumerate(KS):
            sa, ca = at[k]
            for trig in (0, 1):
                src = sa if trig == 0 else ca
                rhs = wcos[k] if trig == 0 else wsin[k]
                for c in range(NCH):
                    half = 0 if c < 8 else 1
                    lo = half * ML
                    nc.tensor.matmul(
                        ps_sT[:, c * 128:(c + 1) * 128],
                        src[:, c * 128:(c + 1) * 128],
                        rhs[:, lo:lo + ML],
                        start=(ki == 0 and trig == 0 and c % 4 == 0),
                        stop=(ki == len(KS) - 1 and trig == 1 and c % 4 == 3),
                        skip_group_check=True)

        # ---- epilogue: softmax + ctx + final linear + store ----
        sb_E = bpool.tile([128, NT], BF16)
        nc.scalar.activation(sb_E[:, 0:N_S], ps_sT[:, 0:N_S], AF.Exp)
        nc.scalar.activation(sb_E[:, N_S:NT], ps_sT[:, N_S:NT], AF.Exp)

        ps_ctx = ps_small.tile([128, 2 * CW], F32, tag="ctx", name="ps_ctx")
        for c in range(8):
            nc.tensor.matmul(ps_ctx[:, 0:CW],
                             sb_E[:, c * 128:(c + 1) * 128],
                             sb_x16[:, c * CW:(c + 1) * CW],
                             start=(c == 0), stop=(c == 7))
        for c in range(8, 12):
            nc.tensor.matmul(ps_ctx[:, CW:2 * CW],
                             sb_E[:, c * 128:(c + 1) * 128],
                             sb_x16[:, c * CW:(c + 1) * CW],
                             start=(c == 8), stop=(c == 11))

        sb_recip = bpool.tile([128, 2], F32)
        nc.vector.reciprocal(sb_recip[:, 0:1], ps_ctx[:, H:H + 1])
        nc.vector.reciprocal(sb_recip[:, 1:2], ps_ctx[:, CW + H:CW + H + 1])
        sb_ctx = bpool.tile([128, 2 * H], F32)
        nc.vector.tensor_scalar_mul(sb_ctx[:, 0:H], ps_ctx[:, 0:H],
                                    sb_recip[:, 0:1])
        nc.vector.tensor_scalar_mul(sb_ctx[:, H:2 * H], ps_ctx[:, CW:CW + H],
                                    sb_recip[:, 1:2])

        # transpose ctx halves [m, h] -> [h, m] via PE, copy to f32r
        ps_tr = ps_a.tile([128, NT], F32, tag="aT", name="ps_tr")
        nc.tensor.matmul(ps_tr[:, 0:128], sb_ctx[:, 0:H], ident[:],
                         is_transpose=True)
        nc.tensor.matmul(ps_tr[:, 512:640], sb_ctx[:, H:2 * H], ident[:],
                         is_transpose=True)
        sb_ctxT = bpool.tile([128, 2 * H], F32R)
        nc.vector.tensor_copy(sb_ctxT[:, 0:H], ps_tr[:, 0:128])
        nc.vector.tensor_copy(sb_ctxT[:, H:2 * H], ps_tr[:, 512:640])

        nc.tensor.matmul(ps_out[:], sb_ctxT[:, 0:H], sb_wlinT[:, A:2 * A],
                         start=False, stop=False, skip_group_check=True)
        nc.tensor.matmul(ps_out[:], sb_ctxT[:, H:2 * H], sb_wlinT[:, 2 * A:3 * A],
                         start=False, stop=True, skip_group_check=True)

        sb_out = bpool.tile([128, A], F32)
        nc.scalar.activation(sb_out[:], ps_out[:], AF.Tanh)
        nc.sync.dma_start(d_out[:, :], sb_out[:])


def _get_nc():
    if "nc" not in _CACHE:
        _CACHE["nc"] = _build()
    return _CACHE["nc"]


def _prep_inputs(inputs):
    """Host-side layout prep: transposes / bf16 casts / SBUF-image packing."""
    f = {k: np.ascontiguousarray(np.asarray(v, np.float32))
         for k, v in inputs.items()}
    stmts, eres = f["attendee_stmts"], f["attendee_eres"]
    ws, we, wlin = f["Ws_concat"], f["We_concat"], f["W_lin"]

    # x image: chunk c holds attendees [c*128, (c+1)*128) as [n_local, h],
    # plus a trailing ones column (turns the ctx matmul into ctx|sum)
    x = np.empty((128, NCH * CW), np.float32)
    for c in range(8):
        x[:, c * CW:c * CW + H] = stmts[c * 128:(c + 1) * 128]
        x[:, c * CW + H] = 1.0
    for c in range(8, 12):
        x[:, c * CW:c * CW + H] = eres[(c - 8) * 128:(c - 7) * 128]
        x[:, c * CW + H] = 1.0
    vb = np.ascontiguousarray(
        np.stack([f["vs_single"], f["ve_single"], f["bs_concat"],
                  f["be_concat"]], axis=1))
    shared = {
        "x16": np.ascontiguousarray(x.astype(bfloat16)),
        "stmtsT": np.ascontiguousarray(stmts.T.astype(bfloat16)),
        "eresT": np.ascontiguousarray(eres.T.astype(bfloat16)),
        "wT16": np.ascontiguousarray(np.concatenate(
            [ws[:, :H].T, ws[:, H:].T, we[:, :H].T, we[:, H:].T],
            axis=1).astype(bfloat16)),
        "wlinT": np.ascontiguousarray(np.concatenate(
            [wlin[:, 0:H].T, wlin[:, H:2 * H].T, wlin[:, 2 * H:3 * H].T],
            axis=1)),
        "vb": vb,
        "blin": np.ascontiguousarray(f["b_lin"][None, :]),
    }
    att = f["attender"]
    in_maps = []
    for i in range(NC):
        attT = np.ascontiguousarray(att[i * ML:(i + 1) * ML].T)
        in_maps.append(dict(shared, attTf=attT,
                            attT16=np.ascontiguousarray(attT.astype(bfloat16))))
    return in_maps


def kernel(**inputs) -> np.ndarray:
    nc = _get_nc()
    in_maps = _prep_inputs(inputs)
    res = run_bass_kernel_spmd(nc, in_maps, list(range(NC)))
    return np.concatenate(
        [res.results[i]["out"].astype(np.float32) for i in range(NC)], axis=0)
